# revision 10
# baseline (speedup 1.0000x reference)
"""Bit-packed binary (masked-XNOR popcount) matmul on 8 TRN2 NeuronCores.

Math: for plane sign s, mask m (bits), the reference computes
    acc[p,b,o] = sum_k popcount(~(x^s) & m)
              = C[p,o] + sum_k x_bit[b,k] * W[p,k,o]
with W = m*(2s-1) in {-1,0,+1} and C[p,o] = sum_k m*(1-s).

Strategy: shard the population axis P=16 across 8 cores (2 each).
Host unpacks w into fp8_e4m3 weights W (exact for {-1,0,1}), x into fp8
{0,1}; each core runs a DoubleRow fp8 PE matmul accumulating exactly in
fp32 PSUM; C is added on the host after gathering.

Layout (per core):
  x host  [kk=128, kcp=16, j=2, b=128]          (k = kcp*256 + j*128 + kk)
  w host  [pl=2, h=2, chunk=4, kk=128, g=4, j=2, col=2048]
          (o = h*2048 + col, kcp = chunk*4 + g)
  One DMA per (pl,h,chunk) moves a contiguous 2MB block with 16KB
  contiguous runs per SBUF partition.
"""

import numpy as np
import ml_dtypes

# Problem dims (hardcoded per contest contract)
B = 128          # batch
I = 64           # packed int64 words per row
K = 4096         # in_features = I*64
O = 4096         # out_features
P = 16           # population
NCORES = 8
PL = P // NCORES   # pop members per core = 2
KCP = 16           # DoubleRow k-pair chunks (256 k each)
OH = 2             # output halves (PSUM capacity)
OHW = O // OH      # 2048
NSUB = OHW // 512  # 512-wide matmul blocks per half = 4
G = 4              # kcp per DMA chunk
NCHUNK = KCP // G  # 4

F8 = ml_dtypes.float8_e4m3

_CACHE = {}


def _build_nc():
    import concourse.bass as bass
    import concourse.mybir as mybir
    import concourse.tile as tile
    from concourse import bacc

    fp8 = mybir.dt.float8e4
    f32 = mybir.dt.float32

    nc = bacc.Bacc("TRN2", target_bir_lowering=False)
    xt_d = nc.dram_tensor("xt", [128, KCP, 2, B], fp8, kind="ExternalInput")
    w_d = nc.dram_tensor(
        "wf", [PL, OH, NCHUNK, 128, G, 2, OHW], fp8, kind="ExternalInput"
    )
    out_d = nc.dram_tensor("out", [PL, OH, B, OHW], f32, kind="ExternalOutput")

    with tile.TileContext(nc) as tc:
        with (
            tc.tile_pool(name="xp", bufs=1) as xp,
            tc.tile_pool(name="wp", bufs=8) as wp,
            tc.tile_pool(name="pp", bufs=2, space=bass.MemorySpace.PSUM) as pp,
            tc.tile_pool(name="op", bufs=2) as op,
        ):
            xt = xp.tile([128, KCP, 2, B], fp8)
            # contiguous 0.5MB load on the fast HWDGE ring, ahead of W
            nc.sync.dma_start(xt[:], xt_d[:])
            dma_engines = [nc.scalar, nc.sync]
            n_dma = 0
            first = True
            for p in range(PL):
                for h in range(OH):
                    ps = pp.tile([128, OHW], f32)
                    for c in range(NCHUNK):
                        wt = wp.tile([128, G, 2, OHW], fp8)
                        # sub-split the chunk DMA so matmuls can start on
                        # earlier g-slices (Tile deps are AP-region level);
                        # split the very first chunk finest for fast start.
                        nsplit = G if first else 2
                        first = False
                        gs = G // nsplit
                        for s in range(nsplit):
                            eng = dma_engines[n_dma % 2]
                            n_dma += 1
                            eng.dma_start(
                                wt[:, s * gs:(s + 1) * gs],
                                w_d[p, h, c, :, s * gs:(s + 1) * gs],
                            )
                        for g in range(G):
                            kcp = c * G + g
                            for oc in range(NSUB):
                                nc.tensor.matmul(
                                    ps[:, oc * 512:(oc + 1) * 512],
                                    xt[:, kcp, :, :],
                                    wt[:, g, :, oc * 512:(oc + 1) * 512],
                                    start=(kcp == 0),
                                    stop=(kcp == KCP - 1),
                                    perf_mode=mybir.MatmulPerfMode.DoubleRow,
                                )
                    ot = op.tile([128, OHW], f32)
                    last_job = (p == PL - 1) and (h == OH - 1)
                    if last_job:
                        # pipeline eviction quarters; HWDGE rings are idle now
                        for q in range(4):
                            sl = slice(q * 512, (q + 1) * 512)
                            nc.vector.tensor_copy(ot[:, sl], ps[:, sl])
                            eng = nc.sync if q % 2 == 0 else nc.scalar
                            eng.dma_start(out_d[p, h, :, sl], ot[:, sl])
                    else:
                        nc.vector.tensor_copy(ot[:], ps[:])
                        nc.gpsimd.dma_start(out_d[p, h], ot[:])

    nc.compile()
    return nc


def _unpack_inputs(x, w):
    """Host-side bit unpack to fp8 operands + popcount bias C."""
    # x bits: [B, K] with k = word*64 + bit (little-endian within words)
    xbits = np.unpackbits(
        np.ascontiguousarray(x).view(np.uint8).reshape(B, I * 8),
        axis=1, bitorder="little",
    )  # [B, K] in {0,1}
    # x host layout [kk, kcp, j, b]
    xtt = np.ascontiguousarray(
        xbits.T.reshape(KCP, 2, 128, B).transpose(2, 0, 1, 3)
    ).astype(F8)

    s_words = np.ascontiguousarray(w[0])  # [P, I, O] int64
    m_words = np.ascontiguousarray(w[1])

    wf_all = np.empty((P, OH, NCHUNK, 128, G, 2, OHW), F8)
    C = np.empty((P, O), np.int32)
    for p in range(P):
        sb = np.unpackbits(
            s_words[p].view(np.uint8).reshape(I, O, 8), axis=2, bitorder="little"
        ).transpose(0, 2, 1).reshape(K, O)  # [K, O] {0,1}
        mb = np.unpackbits(
            m_words[p].view(np.uint8).reshape(I, O, 8), axis=2, bitorder="little"
        ).transpose(0, 2, 1).reshape(K, O)
        Wq = (mb.astype(np.int8) * (2 * sb.astype(np.int8) - 1))  # {-1,0,1}
        C[p] = (mb * (1 - sb)).astype(np.int32).sum(axis=0)
        # [K, O] -> [chunk, g, j, kk, h, col] -> [h, chunk, kk, g, j, col]
        wf_all[p] = (
            Wq.astype(np.float32).astype(F8)
            .reshape(NCHUNK, G, 2, 128, OH, OHW)
            .transpose(4, 0, 3, 1, 2, 5)
        )
    return xtt, wf_all, C


def _run(nc, in_maps, trace=False):
    from concourse import bass_utils
    return bass_utils.run_bass_kernel_spmd(
        nc, in_maps, core_ids=list(range(NCORES)), trace=trace
    )


def kernel(x, w, _trace=False, _return_results=False):
    x = np.asarray(x)
    w = np.asarray(w)
    assert x.shape == (B, I) and w.shape == (2, P, I, O)

    xtt, wf_all, C = _unpack_inputs(x, w)

    if "nc" not in _CACHE:
        _CACHE["nc"] = _build_nc()
    nc = _CACHE["nc"]

    in_maps = [
        {"xt": xtt, "wf": np.ascontiguousarray(wf_all[c * PL:(c + 1) * PL])}
        for c in range(NCORES)
    ]
    res = _run(nc, in_maps, trace=_trace)

    out = np.empty((P, B, O), np.int32)
    for c in range(NCORES):
        o = res.results[c]["out"]  # [PL, OH, B, OHW] f32
        for pl in range(PL):
            full = np.concatenate([o[pl, 0], o[pl, 1]], axis=1)  # [B, O]
            out[c * PL + pl] = full.astype(np.int32) + C[c * PL + pl][None, :]
    if _return_results:
        return out, res
    return out
